# revision 41
# baseline (speedup 1.0000x reference)
"""Causal MHA (B=2, S=2048, D=1024, H=16) sharded over 8 NeuronCores.

Sharding: batch x heads. Core c owns batch c//4 and heads 4*(c%4)..4*(c%4)+4
(weight rows 256*(c%4)..+256). Wq/Wk/Wv split column-parallel by head, Wo
row-parallel; the host sums the 4 partial outputs per batch.

Per-core pipeline (matmul inputs fp16, PSUM f32):
  Host pre-lays every tensor in its exact SBUF layout so all DMAs are
  contiguous; DMAs are sliced and emitted in dependency order so the first
  projection matmul starts a few us in.
  QT/KT [128, 2, S] = W-stationary matmuls over xT with RoPE fused
  (pair-swap via P2 matmul + cos/sin DVE ops). V rows live in
  vns[k-tile, head, 0:64]; columns 64:128 stay at the memset value 1.0, so
  the PV matmul (full 128-wide stationary, same cycle count) replicates
  the softmax denominator into PSUM rows 64:128 for free - the epilogue is
  copy + reciprocal_approx_fast + one multiply, no cross-partition
  broadcast. Per (head, 1024-wide q chunk): S^T[k,q] = K-stationary @ Q,
  P = exp(S/8) (no max subtraction; logits are O(1)), causal diag masked
  by an ident.T@mneg accumulation (-240 pre-scale).
  Projection chunks 2-3 and the chunk-0 output projection are emitted as
  PE filler inside the (ACT-exp-bound) attention streams so the PE never
  idles long enough for the HAM clock gate to re-throttle it. Filler
  units are decoupled (PSUM->SBUF copies on DVE, rope rotations deferred
  two units) so no filler matmul head-of-line blocks the PE on the
  exp-laden ACT queue.
"""

import sys
from collections import deque

import numpy as np

sys.path.insert(0, "/opt/trn_rl_repo")

B, S, D, H = 2, 2048, 1024, 16
DK = D // H            # 64
NCORES = 8
CPB = 4                # cores per batch
HPC = H // CPB         # 4 heads per core
DKH = HPC * DK         # 256 local head-dim
NMT = DKH // 128       # 2 partition tiles of the local head dim
THETA = 10000.0
SCALE = 1.0 / float(np.sqrt(DK))

NT = S // 128          # 16 kr tiles
NCH = S // 1024        # 2 q chunks
NXC = 4                # x chunks of 512
XCW = S // NXC         # 512


def _rope_tables():
    pos = np.arange(S, dtype=np.float64)
    dim = np.arange(0, DK, 2, dtype=np.float64)
    inv_freq = 1.0 / THETA ** (dim / DK)
    angle = pos[None, :] * inv_freq[:, None]        # [DK/2, S]
    angle = np.repeat(angle, 2, axis=0)             # [DK, S] interleaved rows
    cos1, sin1 = np.cos(angle), np.sin(angle)
    cosT = np.concatenate([cos1, cos1], axis=0).astype(np.float16)  # [128, S]
    sinT = np.concatenate([sin1, sin1], axis=0).astype(np.float16)
    return cosT, sinT


def _p2t():
    # pair rotation: out[2i] = -in[2i+1]; out[2i+1] = +in[2i], per 64-row head.
    # matmul computes lhsT.T @ rhs, so pass P2^T. [128,128] covers 2 heads.
    p = np.zeros((DK, DK), dtype=np.float32)
    for i in range(DK // 2):
        p[2 * i, 2 * i + 1] = -1.0
        p[2 * i + 1, 2 * i] = 1.0
    p2 = np.zeros((128, 128), dtype=np.float32)
    p2[:DK, :DK] = p
    p2[DK:, DK:] = p
    return np.ascontiguousarray(p2.T).astype(np.float16)


def _mask_tables():
    # ident.T @ mneg accumulated into a diagonal score tile adds -240 where
    # q < k (pre-scale), so exp(s/8 - 30) == 0 for non-causal pairs
    ident = np.eye(128, dtype=np.float16)
    mneg = np.tril(np.full((128, 128), -240.0, dtype=np.float16), -1)
    return ident, mneg


def _col_pieces(qs):
    """Split [qs, 1024) into <=512-wide matmul column pieces."""
    if qs >= 512:
        return [(qs, 1024)]
    return [(qs, 512), (512, 1024)]


def _build_nc(debug=False):
    from contextlib import ExitStack

    import concourse.bass as bass  # noqa: F401
    import concourse.tile as tile
    from concourse import bacc, mybir

    NDBG = NCH * HPC

    fp16 = mybir.dt.float16
    bf16 = mybir.dt.bfloat16
    f32 = mybir.dt.float32
    EXP = mybir.ActivationFunctionType.Exp
    MULT = mybir.AluOpType.mult

    nc = bacc.Bacc(
        "TRN2", target_bir_lowering=False, debug=False, num_devices=NCORES
    )
    # all inputs pre-laid-out by the host in exact SBUF order -> contiguous DMA
    xt_d = nc.dram_tensor("xT", [NXC, 128, 8 * XCW], fp16, kind="ExternalInput")
    wqt_d = nc.dram_tensor("wqt", [128, 8 * DKH], fp16, kind="ExternalInput")
    wkt_d = nc.dram_tensor("wkt", [128, 8 * DKH], fp16, kind="ExternalInput")
    wvt_d = nc.dram_tensor("wvt", [128, 8 * DKH], fp16, kind="ExternalInput")
    wot_d = nc.dram_tensor("wot", [128, NMT * D], fp16, kind="ExternalInput")
    cos_d = nc.dram_tensor("cosT", [128, S], fp16, kind="ExternalInput")
    sin_d = nc.dram_tensor("sinT", [128, S], fp16, kind="ExternalInput")
    p2t_d = nc.dram_tensor("p2t", [128, 128], fp16, kind="ExternalInput")
    ident_d = nc.dram_tensor("ident", [128, 128], fp16, kind="ExternalInput")
    mneg_d = nc.dram_tensor("mneg", [128, 128], fp16, kind="ExternalInput")
    # fp8 copies of x chunks 2-3 + Wv for DoubleRow V-projection (B1-h0's
    # filler region is PE-bound; 1.44x there is a straight wall-clock win)
    f8 = mybir.dt.float8e4
    xt8_d = nc.dram_tensor("xT8", [2, 128, 8 * XCW], f8, kind="ExternalInput")
    wvt8_d = nc.dram_tensor("wvt8", [128, 8 * DKH], f8, kind="ExternalInput")
    out_d = nc.dram_tensor("out", [S, D], fp16, kind="ExternalOutput")
    if debug:
        dbg_q = nc.dram_tensor("dbg_q", [128, NMT, S], fp16, kind="ExternalOutput")
        dbg_k = nc.dram_tensor("dbg_k", [128, NMT, S], fp16, kind="ExternalOutput")
        dbg_v = nc.dram_tensor(
            "dbg_v", [128, NT * HPC * 128], bf16, kind="ExternalOutput"
        )
        dbg_den = nc.dram_tensor(
            "dbg_den", [NDBG, 64, 1024], f32, kind="ExternalOutput"
        )
        dbg_rdb = nc.dram_tensor(
            "dbg_rdb", [NDBG, 64, 1024], f32, kind="ExternalOutput"
        )
        dbg_st = nc.dram_tensor("dbg_st", [128, NMT, S], fp16, kind="ExternalOutput")

    with tile.TileContext(nc) as tc, ExitStack() as ctx:
        consts = ctx.enter_context(tc.tile_pool(name="consts", bufs=1))
        xtp = ctx.enter_context(tc.tile_pool(name="xt", bufs=1))
        qkp = ctx.enter_context(tc.tile_pool(name="qk", bufs=1))
        vnp = ctx.enter_context(tc.tile_pool(name="vn", bufs=1))
        stackp = ctx.enter_context(tc.tile_pool(name="stack", bufs=1))
        rawp = ctx.enter_context(tc.tile_pool(name="raw", bufs=3))
        tmpp = ctx.enter_context(tc.tile_pool(name="tmp", bufs=4))
        ptp = ctx.enter_context(tc.tile_pool(name="pt", bufs=6))
        epip = ctx.enter_context(tc.tile_pool(name="epi", bufs=2))
        outp = ctx.enter_context(tc.tile_pool(name="outsb", bufs=5))

        # ---- tiles ----
        p2_sb = consts.tile([128, 128], fp16, tag="p2")
        wk_sb = consts.tile([128, 8, DKH], fp16, tag="wk")
        wq_sb = consts.tile([128, 8, DKH], fp16, tag="wq")
        wv_sb = consts.tile([128, 8, DKH], fp16, tag="wv")
        cos_sb = consts.tile([128, S], fp16, tag="cos")
        sin_sb = consts.tile([128, S], fp16, tag="sin")
        xt = xtp.tile([128, NXC, 8, XCW], fp16, tag="xt")
        ident_sb = consts.tile([128, 128], fp16, tag="ident")
        mneg_sb = consts.tile([128, 128], fp16, tag="mneg")
        wo_sb = consts.tile([128, NMT, D], fp16, tag="wo")

        # ---- DMA emission = dependency order, sliced in j-pairs so the
        # first projection matmuls start early (queues drain in order) ----
        wk_src = wkt_d[:, :].rearrange("p (j m) -> p j m", j=8)
        wq_src = wqt_d[:, :].rearrange("p (j m) -> p j m", j=8)
        wv_src = wvt_d[:, :].rearrange("p (j m) -> p j m", j=8)
        xt_srcs = [
            xt_d[c, :, :].rearrange("p (j s) -> p j s", j=8) for c in range(NXC)
        ]
        nc.sync.dma_start(p2_sb, p2t_d[:, :])
        for j0 in range(0, 8, 2):
            jsl = slice(j0, j0 + 2)
            nc.sync.dma_start(wk_sb[:, jsl], wk_src[:, jsl])
            nc.sync.dma_start(xt[:, 0, jsl], xt_srcs[0][:, jsl])
            if j0 == 2:
                nc.sync.dma_start(cos_sb[:, 0:XCW], cos_d[:, 0:XCW])
                nc.sync.dma_start(sin_sb[:, 0:XCW], sin_d[:, 0:XCW])
        for j0 in range(0, 8, 2):
            nc.sync.dma_start(wv_sb[:, j0 : j0 + 2], wv_src[:, j0 : j0 + 2])
        for j0 in range(0, 8, 2):
            nc.sync.dma_start(wq_sb[:, j0 : j0 + 2], wq_src[:, j0 : j0 + 2])
        nc.sync.dma_start(ident_sb, ident_d[:, :])
        nc.sync.dma_start(mneg_sb, mneg_d[:, :])
        for j0 in range(0, 8, 2):
            nc.sync.dma_start(xt[:, 1, j0 : j0 + 2], xt_srcs[1][:, j0 : j0 + 2])
        nc.sync.dma_start(cos_sb[:, XCW : 2 * XCW], cos_d[:, XCW : 2 * XCW])
        nc.sync.dma_start(sin_sb[:, XCW : 2 * XCW], sin_d[:, XCW : 2 * XCW])
        xt8 = xtp.tile([128, 2, 8, XCW], f8, tag="xt8")
        wv8_sb = consts.tile([128, 8, DKH], f8, tag="wv8")
        nc.sync.dma_start(
            wv8_sb, wvt8_d[:, :].rearrange("p (j m) -> p j m", j=8)
        )
        for cch in (2, 3):
            for j0 in range(0, 8, 2):
                nc.sync.dma_start(
                    xt[:, cch, j0 : j0 + 2], xt_srcs[cch][:, j0 : j0 + 2]
                )
            nc.sync.dma_start(
                xt8[:, cch - 2],
                xt8_d[cch - 2, :, :].rearrange("p (j s) -> p j s", j=8),
            )
        nc.sync.dma_start(cos_sb[:, 2 * XCW :], cos_d[:, 2 * XCW :])
        nc.sync.dma_start(sin_sb[:, 2 * XCW :], sin_d[:, 2 * XCW :])
        nc.sync.dma_start(wo_sb, wot_d[:, :].rearrange("p (k m) -> p k m", k=NMT))

        qtr = qkp.tile([128, NMT, S], fp16, tag="qtr")
        ktr = qkp.tile([128, NMT, S], fp16, tag="ktr")
        # V rows in cols 0:64; cols 64:128 stay 1.0 so the PV matmul writes
        # the softmax denominator, already broadcast, into PSUM rows 64:128
        vns = vnp.tile([128, NT, HPC, 128], bf16, tag="vn")
        nc.vector.memset(vns[:, :, :, :], 1.0)
        stackT = stackp.tile([128, NMT, S], fp16, tag="stack")

        # PSUM: psA 2x[128,512] (2 banks) + sc 2x[128,1024] (4) + at (2) = 8
        psA = ctx.enter_context(tc.tile_pool(name="psA", bufs=2, space="PSUM"))
        ps_sc = ctx.enter_context(tc.tile_pool(name="ps_sc", bufs=2, space="PSUM"))
        ps_at = ctx.enter_context(tc.tile_pool(name="ps_at", bufs=1, space="PSUM"))

        # ================= phase A units =================
        raw_state = {}

        def proj_group(w_sb, cch, mt, key, on_act):
            # one [128, 512] projection accumulation; PSUM->SBUF copy on
            # ACT in projection-only phases, on DVE when used as filler
            # inside the exp-bound attention stream (fast turnaround)
            ps = psA.tile([128, XCW], f32, tag="ps")
            for j in range(8):
                nc.tensor.matmul(
                    ps,
                    w_sb[:, j, 128 * mt : 128 * (mt + 1)],
                    xt[:, cch, j, :],
                    start=(j == 0),
                    stop=(j == 7),
                )
            raw = rawp.tile([128, XCW], fp16, tag="raw")
            if on_act:
                nc.scalar.copy(raw, ps)
            else:
                nc.vector.tensor_copy(raw, ps)
            raw_state[key] = raw

        def rope_rot(dst, cch, mt, key):
            raw = raw_state.pop(key)
            sl = slice(XCW * cch, XCW * (cch + 1))
            rot_ps = psA.tile([128, XCW], f32, tag="ps")
            nc.tensor.matmul(rot_ps, p2_sb, raw, start=True, stop=True)
            t1 = tmpp.tile([128, XCW], fp16, tag="tmp")
            nc.vector.tensor_mul(t1, raw, cos_sb[:, sl])
            t2 = tmpp.tile([128, XCW], fp16, tag="tmp")
            nc.vector.tensor_tensor(t2, rot_ps, sin_sb[:, sl], op=MULT)
            nc.vector.tensor_add(dst[:, mt, sl], t1, t2)

        def proj_chunk(w_sb, dst, cch, on_act=True):
            proj_group(w_sb, cch, 0, (cch, 0), on_act)
            proj_group(w_sb, cch, 1, (cch, 1), on_act)
            rope_rot(dst, cch, 0, (cch, 0))
            rope_rot(dst, cch, 1, (cch, 1))

        def v_pair(cch, sp):
            for st in (2 * sp, 2 * sp + 1):
                t_g = 4 * cch + st
                vps = psA.tile([128, DKH], f32, tag="ps")
                if cch >= 2:
                    # fp8 DoubleRow: j-pairs contract 256-deep per matmul
                    # (x rows d = 256*jp + 128*i + p pair with Wv rows)
                    for jp in range(4):
                        nc.tensor.matmul(
                            vps,
                            xt8[
                                :, cch - 2, 2 * jp : 2 * jp + 2,
                                128 * st : 128 * (st + 1),
                            ],
                            wv8_sb[:, 2 * jp : 2 * jp + 2, :],
                            start=(jp == 0),
                            stop=(jp == 3),
                            perf_mode=mybir.MatmulPerfMode.DoubleRow,
                        )
                else:
                    for j in range(8):
                        nc.tensor.matmul(
                            vps,
                            xt[:, cch, j, 128 * st : 128 * (st + 1)],
                            wv_sb[:, j, :],
                            start=(j == 0),
                            stop=(j == 7),
                        )
                nc.vector.tensor_copy(
                    vns[:, t_g, :, 0:64],
                    vps[:, :].rearrange("p (h d) -> p h d", h=HPC),
                )

        def oproj(qt_i, tail=False):
            # tail variant splits the copy across ACT+DVE and DMAs halves
            osb = outp.tile([128, D], fp16, tag="osb")
            for oc in range(2):
                po = psA.tile([128, 512], f32, tag="ps")
                for mt2 in range(NMT):
                    nc.tensor.matmul(
                        po,
                        stackT[:, mt2, 128 * qt_i : 128 * (qt_i + 1)],
                        wo_sb[:, mt2, 512 * oc : 512 * (oc + 1)],
                        start=(mt2 == 0),
                        stop=(mt2 == NMT - 1),
                    )
                half = osb[:, 512 * oc : 512 * (oc + 1)]
                if tail and oc == 0:
                    nc.scalar.copy(half, po)
                else:
                    nc.vector.tensor_copy(half, po)
                if tail:
                    nc.sync.dma_start(
                        out_d[
                            128 * qt_i : 128 * (qt_i + 1),
                            512 * oc : 512 * (oc + 1),
                        ],
                        half,
                    )
            if not tail:
                nc.sync.dma_start(out_d[128 * qt_i : 128 * (qt_i + 1), :], osb)

        # ================= phase B =================
        def bhead(cch, h, fillers, pop_mod=1, last=False):
            qbase = 1024 * cch
            n_kt = min(NT, 8 * (cch + 1))
            mt = h // 2
            hsl = slice(64 * (h % 2), 64 * (h % 2) + 64)
            at_ps = ps_at.tile([128, 1024], f32, tag="at")

            def epilogue(lo, hi, on_act):
                # PSUM rows 64:128 hold the denominator already broadcast
                # across partitions (ones-columns of vns); recip needs an
                # SBUF input (PSUM-direct is broken on HW)
                den_sb = epip.tile([64, hi - lo], f32, tag="den")
                if on_act:
                    nc.scalar.copy(den_sb, at_ps[64:128, lo:hi])
                else:
                    nc.vector.tensor_copy(den_sb, at_ps[64:128, lo:hi])
                rdb = epip.tile([64, hi - lo], f32, tag="rdb")
                nc.vector.reciprocal_approx_fast(out=rdb, in_=den_sb)
                nc.vector.tensor_tensor(
                    stackT[hsl, mt, qbase + lo : qbase + hi],
                    at_ps[0:64, lo:hi],
                    rdb,
                    op=MULT,
                )

            def emit_pv(t, pt, qs):
                # PSUM accumulation-group granularity is the 2KB bank
                # (512 f32 cols); stop lands on each bank's diagonal tile
                for lo, hi in _col_pieces(qs):
                    nc.tensor.matmul(
                        at_ps[:, lo:hi],
                        vns[:, t, h, :],
                        pt[:, lo:hi],
                        start=(t == 0),
                        stop=(qs == hi - 128),
                    )

            pend = deque()  # software-pipeline PV two kr-tiles back
            for t in range(n_kt):
                qs = max(128 * t - qbase, 0)
                diag = 128 * t >= qbase
                sc_ps = ps_sc.tile([128, 1024], f32, tag="sc")
                for lo, hi in _col_pieces(qs):
                    nc.tensor.matmul(
                        sc_ps[:, lo:hi],
                        ktr[hsl, mt, 128 * t : 128 * (t + 1)],
                        qtr[hsl, mt, qbase + lo : qbase + hi],
                        start=True,
                        stop=not (diag and lo == qs),
                    )
                if diag:
                    # causal mask: add -240 (pre-scale) where q < k
                    nc.tensor.matmul(
                        sc_ps[:, qs : qs + 128],
                        ident_sb,
                        mneg_sb,
                        start=False,
                        stop=True,
                    )
                pt = ptp.tile([128, 1024], bf16, tag="pt")
                # bf16 P: fp32-range exponent, exp cannot overflow
                nc.scalar.activation(
                    pt[:, qs:1024], sc_ps[:, qs:1024], EXP, scale=SCALE
                )
                pend.append((t, pt, qs))
                if len(pend) > 2:
                    emit_pv(*pend.popleft())
                if last and t == 13:
                    # cols 0:512 saw their final PV (tile 11) above; emit
                    # that half's epilogue now so the tail output
                    # projection overlaps the last tiles + second half
                    epilogue(0, 512, on_act=False)
                if fillers and t % pop_mod == pop_mod - 1:
                    fillers.popleft()()
            while pend:
                emit_pv(*pend.popleft())
            if last:
                epilogue(512, 1024, on_act=True)  # ACT idle at the tail
            else:
                epilogue(0, 1024, on_act=False)

        # ================= emission schedule =================
        # critical prefix: the minimum attention-h0 needs up front (K/V/Q
        # for x chunk 0 + Q chunk 1), ordered for the smallest DMA prefix
        # per unit; everything else rides as filler, each unit emitted
        # before the attention k-tile that first reads its output. This
        # keeps the PE saturated end-to-end - an idling PE gets clock-
        # gated to 1.2 GHz by the HAM and then becomes the bottleneck.
        proj_chunk(wk_sb, ktr, 0)
        for sp in range(2):
            v_pair(0, sp)
        proj_chunk(wq_sb, qtr, 0)
        proj_chunk(wq_sb, qtr, 1)

        def proj_units(w_sb, dst, nm, cch):
            return [
                lambda: proj_group(w_sb, cch, 0, (nm, cch, 0), False),
                lambda: proj_group(w_sb, cch, 1, (nm, cch, 1), False),
                lambda: rope_rot(dst, cch, 0, (nm, cch, 0)),
                lambda: rope_rot(dst, cch, 1, (nm, cch, 1)),
            ]

        # h0 carries K1/V1 (K1 rot done by t=2, first read t=4; V1 by
        # t=5, first read by the lagged PV at t=6)
        fillkv = deque(
            proj_units(wk_sb, ktr, "k", 1)
            + [lambda: v_pair(1, 0), lambda: v_pair(1, 1)]
        )
        bhead(0, 0, fillkv)
        while fillkv:
            fillkv.popleft()()
        fill0 = deque(
            proj_units(wq_sb, qtr, "q", 2) + proj_units(wq_sb, qtr, "q", 3)
        )
        for h in range(1, HPC):
            bhead(0, h, fill0, pop_mod=3)
        while fill0:
            fill0.popleft()()

        # attention chunk 1: h0 carries K2/K3/V2/V3 (each ready before the
        # k-tile that first reads it), h1-h3 the chunk-0 output projection
        filla = deque(
            proj_units(wk_sb, ktr, "k", 2)
            + [lambda: v_pair(2, 0), lambda: v_pair(2, 1)]
            + proj_units(wk_sb, ktr, "k", 3)
            + [lambda: v_pair(3, 0), lambda: v_pair(3, 1)]
        )
        bhead(1, 0, filla, pop_mod=1)
        while filla:
            filla.popleft()()
        fillb = deque(lambda q=qt_i: oproj(q) for qt_i in range(8))
        for h in range(1, HPC):
            bhead(1, h, fillb, pop_mod=5, last=(h == HPC - 1))
        while fillb:
            fillb.popleft()()
        for qt_i in range(8, 16):
            oproj(qt_i, tail=True)
        if debug:
            nc.sync.dma_start(dbg_q[:, :, :], qtr[:, :, :])
            nc.sync.dma_start(dbg_k[:, :, :], ktr[:, :, :])
            nc.sync.dma_start(
                dbg_v[:, :], vns[:, :, :, :].rearrange("p a b c -> p (a b c)")
            )
            nc.sync.dma_start(dbg_st[:, :, :], stackT[:, :, :])

    nc.compile()
    return nc


_NC_CACHE = None


def _in_maps(x, Wq, Wk, Wv, Wo):
    cosT, sinT = _rope_tables()
    p2t = _p2t()
    ident, mneg = _mask_tables()
    Wq, Wk, Wv, Wo = (np.asarray(w, dtype=np.float32) for w in (Wq, Wk, Wv, Wo))
    x = np.asarray(x, dtype=np.float32)
    # x^T chunk-major: [cch, p, j, s] so every DMA is fully contiguous
    xts = []
    for b in range(B):
        xt = x[b].T.astype(np.float16)                       # [D, S]
        xt = xt.reshape(8, 128, NXC, XCW).transpose(2, 1, 0, 3)
        xts.append(np.ascontiguousarray(xt.reshape(NXC, 128, 8 * XCW)))

    def wlay(w):  # [D, DKH] -> [128, 8*DKH] in (p, j, m) order
        w = w.reshape(8, 128, DKH).transpose(1, 0, 2)
        return np.ascontiguousarray(w.reshape(128, 8 * DKH).astype(np.float16))

    import ml_dtypes

    f8 = ml_dtypes.float8_e4m3fn  # bitwise == TRN fp8e4 for |v| <= 240
    xts8 = [np.ascontiguousarray(xt[2:4].astype(f8)) for xt in xts]

    in_maps = []
    for c in range(NCORES):
        b = c // CPB
        rows = slice(DKH * (c % CPB), DKH * (c % CPB + 1))
        wol = Wo[:, rows].T.reshape(NMT, 128, D).transpose(1, 0, 2)
        in_maps.append(
            {
                "xT": xts[b],
                "xT8": xts8[b],
                "wvt8": wlay(Wv[rows, :].T).astype(f8),
                "wqt": wlay(Wq[rows, :].T),
                "wkt": wlay(Wk[rows, :].T),
                "wvt": wlay(Wv[rows, :].T),
                "wot": np.ascontiguousarray(
                    wol.reshape(128, NMT * D).astype(np.float16)
                ),
                "cosT": cosT,
                "sinT": sinT,
                "p2t": p2t,
                "ident": ident,
                "mneg": mneg,
            }
        )
    return in_maps


def kernel(x, Wq, Wk, Wv, Wo):
    global _NC_CACHE
    from concourse.bass_utils import run_bass_kernel_spmd

    if _NC_CACHE is None:
        _NC_CACHE = _build_nc()
    nc = _NC_CACHE

    in_maps = _in_maps(x, Wq, Wk, Wv, Wo)
    res = run_bass_kernel_spmd(nc, in_maps, core_ids=list(range(NCORES)))
    out = np.zeros((B, S, D), dtype=np.float32)
    for c, r in enumerate(res.results):
        out[c // CPB] += r["out"].astype(np.float32)
    return out
